# revision 39
# baseline (speedup 1.0000x reference)
"""Trainium2 Bass kernel for nn_Attention_5927054869144.

Channel-attention over [B=8, C=64, H=256, W=256] inputs. Data-parallel over
batch: one batch element per NeuronCore (8 cores), no collectives.

Per-core pipeline (x_b viewed as [64, 65536], spatial blocks of 8192):
  1. qkvT projection with x-chunk stationary on the PE -> q/k/v directly in
     spatial-partition layout (fp16 operands, fp32 PSUM). Four projection
     matmuls share one 2-bank PSUM tile (256-col slots) so each tile is
     evacuated with a single strided copy (halves the DVE/ACT fixed cost).
  2. Per-head-pair dots matmuls from gathered (alpha, i) column APs,
     accumulated in PSUM over all spatial tiles (softmax scale folded into
     Wq/bq host-side).
  3. v transposed to dim-partition layout via TensorE gather-transposes
     (fp16 PSUM, one single-bank tile ping-pongs two groups so PSUM stays
     within 8 banks). (xbar DMA-transpose was tried and is 2x WORSE: the
     framework serializes DMA_TRANSPOSE against every other in-flight DMA
     as a HW-deadlock guard, lock-stepping the x-load stream.)
  4. Unnormalized softmax: exp(x - max) on ScalarE with accumulated row sums;
     1/rowsum folded into per-head copies of Wo^T.
  5. M_h^T = expdots_h @ (Wo^T * recip) via tiny matmuls; final output =
     M_h^T.T @ v_dp, evacuated fp32->fp16 and stored per 512-col chunk so the
     last store tail is ~3us instead of ~16us.

Perf notes: PE warm-up matmuls keep the HAM clock gate at 2.4 GHz from the
first projection; quarter-0 x tiles are loaded half-major across the four
block-pair tiles so the first projection group is runnable after ~1MB of DMA.
"""

import os
import sys

import numpy as np

for _p in ("/opt/trn_rl_repo", "/root/.axon_site/_ro/trn_rl_repo"):
    if os.path.isdir(_p) and _p not in sys.path:
        sys.path.insert(0, _p)

from concourse import bacc, mybir, tile  # noqa: E402
from concourse.bass_utils import run_bass_kernel_spmd  # noqa: E402

F32 = mybir.dt.float32
F16 = mybir.dt.float16
BF16 = mybir.dt.bfloat16

HEADS = 8
C = 64
HW = 65536          # 256*256 spatial positions per batch element
BL = HW // HEADS    # 8192, per-head block length
NQ = 4              # spatial quarters (within-block n ranges)
QL = BL // NQ       # 2048 within-block positions per quarter
TPQ = QL // 128     # 16 tile groups per quarter
N_GROUPS = BL // 128  # 64 total tile groups
N_WARM = 6          # N=512 PE warm-up matmuls (~2.6us cold) before first x tile

LAST_RESULTS = None


def _build_kernel(hw=HW):
    bl = hw // HEADS
    ql = bl // NQ
    tpq = ql // 128
    n_groups = bl // NQ // 128 * NQ

    nc = bacc.Bacc("TRN2", target_bir_lowering=False, debug=False)
    x_d = nc.dram_tensor("x", [65, hw], F32, kind="ExternalInput")
    wqkv_d = nc.dram_tensor("wqkv", [65, 192], F16, kind="ExternalInput")
    wot_d = nc.dram_tensor("wot", [128, 64], F32, kind="ExternalInput")
    ident_d = nc.dram_tensor("ident", [128, 128], F16, kind="ExternalInput")
    # fp16 output in evac-native layout: row s*64+o holds, for each pair pr,
    # block 2*pr+s of channel o at cols pr*bl. Host decodes + casts to fp32.
    out_d = nc.dram_tensor("out", [128, hw // 2], F16, kind="ExternalOutput")

    x_ap = x_d.ap()
    out_ap = out_d.ap()

    with tile.TileContext(nc) as tc:
        with (
            tc.tile_pool(name="consts", bufs=1) as cpool,
            tc.tile_pool(name="pers", bufs=1) as pers,
            tc.tile_pool(name="dotsp", bufs=1, space="PSUM") as dotspool,
        ):
            wqkv_sb = cpool.tile([65, 192], F16)
            wot_sb = cpool.tile([128, 64], F32)
            ident_sb = cpool.tile([128, 128], F16)
            nc.sync.dma_start(out=wqkv_sb[:, :], in_=wqkv_d.ap()[:, :])
            nc.sync.dma_start(out=wot_sb[:, :], in_=wot_d.ap()[:, :])
            nc.sync.dma_start(out=ident_sb[:, :], in_=ident_d.ap()[:, :])

            # v in dim-partition layout: [pair, d(0:64 even head / 64:128 odd), n]
            vdp = pers.tile([128, 4 * bl], F16)
            # all 4 pairs' dots share one PSUM bank. Never use start=True
            # here: a start's whole-bank has_written clear can race the
            # neighboring pairs' first drains (observed intermittent g0 loss
            # under shifted timing). Instead zero the bank once via DVE and
            # let every matmul accumulate / overwrite-on-clear-bits.
            dots_big = dotspool.tile([128, 512], F32, name="dots")
            nc.vector.memset(dots_big[:, :], 0.0)
            dots_ps = [dots_big[:, 128 * p:128 * p + 128] for p in range(4)]

            # ---------------- Phase A ----------------
            vdp_v = vdp.rearrange("p (r n) -> p r n", r=4)
            with (
                tc.tile_pool(name="xq", bufs=12) as xpool,
                tc.tile_pool(name="slots", bufs=12) as slotpool,
                tc.tile_pool(name="projp", bufs=3, space="PSUM") as projpool,
                tc.tile_pool(name="vtrp", bufs=1, space="PSUM") as vtrpool,
            ):
                # one 2KB fp16 bank ping-pongs two groups' v transposes:
                # group g uses cols (g%2)*512, so group g+2's transposes only
                # wait on group g's evac - two groups of slack from one bank.
                vt_pp = vtrpool.tile([128, 1024], F16, name="vt")
                # PE warm-up during the first x DMA so HAM un-throttles
                # before real work arrives. N=512 streams: small-N warmups
                # (N=64) demonstrably never latch the HAM SHORT window on
                # this silicon (both prior profiles stayed at K=4/8 through
                # phase A); long streams do.
                warm_w = cpool.tile([65, 576], F16)
                nc.vector.memset(warm_w[:, :], 1.0)
                warm_tile = projpool.tile([128, 1024], F32, name="pp")
                warm_ps = warm_tile[0:64, 0:512]
                for _ in range(N_WARM):
                    nc.tensor.matmul(
                        warm_ps[:, :],
                        lhsT=warm_w[:, 0:64],
                        rhs=warm_w[:, 64:576],
                        start=True,
                        stop=True,
                    )

                # deliberate PE power-dip: on this platform the 2.4 GHz
                # grant is withheld by a power-management throttler while
                # the PE runs flat-out; kernels whose pipelines briefly
                # stall get granted K=8/8 right after the dip (baseline
                # latches at its evac-stall ~42us, every run; a perfectly
                # pipelined stream stays at 1.2 GHz forever). Emulate the
                # dip: a serial gpsimd chain the PE must wait on (~3us).
                pause_w = cpool.tile([65, 4096], F16)

                def pe_pause():
                    nc.gpsimd.memset(pause_w[:, :], 0.5)
                    nc.gpsimd.memset(pause_w[:, :], 1.0)
                    pp_pause = projpool.tile([128, 1024], F32, name="pp")
                    nc.tensor.matmul(
                        pp_pause[0:64, 0:64],
                        lhsT=pause_w[:, 0:64],
                        rhs=pause_w[:, 64:128],
                        start=True,
                        stop=True,
                    )

                x_blk = x_ap.rearrange("p (i n) -> p i n", i=8)
                slots = {}
                vt_next = 0

                def consume_dots(g):
                    # dots for a group whose slot is fully evac'd
                    slot = slots[g]
                    for pr in range(4):
                        nc.tensor.matmul(
                            dots_ps[pr][:, :],
                            lhsT=slot[:, 128 * pr: 128 * pr + 128],
                            rhs=slot[:, 512 + 128 * pr: 512 + 128 * pr + 128],
                            start=False,
                            stop=(g == n_groups - 1),
                        )

                def consume_vt(g):
                    # v-transposes. transpose-mode ops don't count as
                    # PE-busy for the HAM activity monitor, so these are
                    # scheduled in a skip-2/catch-up-2 pattern (see below)
                    # leaving periodic transpose-free stretches >3.4us in
                    # which HAM can (re)latch the 2.4 GHz clock state.
                    slot = slots.pop(g)
                    vt = vt_pp[:, (g % 2) * 512:(g % 2) * 512 + 512]
                    for pr in range(4):
                        vs = slot[:, 1024 + 128 * pr: 1024 + 128 * pr + 128]
                        nc.tensor.transpose(
                            vt[:, pr * 128:(pr + 1) * 128], vs, ident_sb[:, :]
                        )
                    if g % 2 == 1:
                        # evacuate both ping-pong halves (2 groups) in one
                        # copy; alternate engines (DVE is ~1.5x faster on
                        # fp16 so the fp32 tile evac split stays balanced)
                        lo = (g - 1) * 128
                        vdst = vdp_v[:, :, lo:lo + 256].rearrange(
                            "p q (t n) -> p q t n", t=2)
                        vsrc = vt_pp.rearrange("p (t q n) -> p q t n", t=2, q=4)
                        if g % 4 == 1:
                            nc.scalar.copy(vdst, vsrc)
                        else:
                            nc.vector.tensor_copy(vdst, vsrc)

                for q in range(NQ):
                    # per-(quarter, block-pair) x tiles: 1 MB DMAs, so the
                    # first projection starts early and later tiles prefetch
                    # behind compute on the gpsimd SWDGE queue.
                    xqt = []
                    nh = 8 if q == 0 else 2
                    hl = ql // nh
                    for ip in range(4):
                        xq = xpool.tile([65, 2 * ql], F16, name="xq")
                        xqt.append(xq)
                    # half-major issue order for the first quarter: all four
                    # tiles' first halves arrive before any second half, so
                    # group 0 is runnable ~3x sooner.
                    for hh in range(nh):
                        for ip in range(4):
                            xq_v = xqt[ip].rearrange("p (c n) -> p c n", c=2)
                            nc.gpsimd.dma_start(
                                out=xq_v[:, :, hh * hl:(hh + 1) * hl],
                                in_=x_blk[:, 2 * ip:2 * ip + 2,
                                          q * ql + hh * hl:
                                          q * ql + (hh + 1) * hl],
                            )
                    for t0 in range(tpq):
                        g = q * tpq + t0
                        if g in (12, 36):
                            pe_pause()
                        # slot cols: r*512 + head*64 + blk*8 + a (a=chan-in-head)
                        slot = slotpool.tile([128, 1536], F16, name="slot")
                        slot_sc = slot.rearrange(
                            "p (r h j a) -> p j r h a", r=3, h=8, j=8, a=8
                        )
                        slots[g] = slot
                        for half in range(2):  # blocks 0-3 / 4-7
                            pp = projpool.tile([128, 1024], F32, name="pp")
                            for jj in range(4):
                                j = 4 * half + jj
                                ip, c = j // 2, j % 2
                                nc.tensor.matmul(
                                    pp[:, jj * 256: jj * 256 + 192],
                                    lhsT=xqt[ip][:, c * ql + t0 * 128:
                                                  c * ql + t0 * 128 + 128],
                                    rhs=wqkv_sb[:, :],
                                    start=(jj % 2 == 0),
                                    stop=(jj % 2 == 1),
                                )
                            # single strided evac per 4-matmul tile:
                            # src [p, j, r, h, a] strides (256, 64, 8, 1)
                            pp_v = pp.rearrange(
                                "p (j r h a) -> p j r h a", j=4, r=4, h=8, a=8
                            )[:, :, 0:3, :, :]
                            dst = slot_sc[:, 4 * half: 4 * half + 4]
                            if half == 0:
                                nc.vector.tensor_copy(dst, pp_v)
                            else:
                                nc.scalar.copy(dst, pp_v)
                        # consume lag 8: the first ~8 groups are a pure
                        # projection-matmul stream with no transpose-mode
                        # ops, giving HAM a clean 4096-cycle busy window to
                        # latch K=8/8 early; afterwards the vt skip/catch-up
                        # pattern re-opens such a window every 8 groups.
                        if g >= 8:
                            consume_dots(g - 8)
                        ph = g % 8
                        nvt = 0 if ph in (6, 7) else (2 if ph in (0, 1) else 1)
                        while nvt > 0 and vt_next <= g - 8:
                            consume_vt(vt_next)
                            vt_next += 1
                            nvt -= 1
                for g in range(n_groups - 8, n_groups):
                    consume_dots(g)
                while vt_next < n_groups:
                    consume_vt(vt_next)
                    vt_next += 1

            # ---------------- Softmax + output ----------------
            with (
                tc.tile_pool(name="smx", bufs=1) as smx,
                tc.tile_pool(name="mhp", bufs=1, space="PSUM") as mhpool,
                tc.tile_pool(name="finp", bufs=3, space="PSUM") as finpool,
                tc.tile_pool(name="outs", bufs=4) as outpool,
            ):
                # bridge the softmax DVE latency with N=512 PE warm-up
                # matmuls so HAM doesn't re-throttle before the output pass
                wbr = finpool.tile([128, 512], F32, name="fp_")
                for _ in range(12):
                    nc.tensor.matmul(
                        wbr[0:64, :],
                        lhsT=warm_w[:, 0:64],
                        rhs=warm_w[:, 64:576],
                        start=True,
                        stop=True,
                    )

                negmax = smx.tile([128, 4], F32)
                rowsum = smx.tile([128, 4], F32)
                recip = smx.tile([128, 4], F32)
                exps = smx.tile([128, 4 * 64], F16)
                wots = smx.tile([128, 4 * 64], F16)
                mh_sb = smx.tile([128, 4 * 64], F16)

                # stage-major softmax (max-sub IS required: dots max ~550)
                # so the per-head cross-engine chains pipeline.
                mh_ps = mhpool.tile([128, 256], F32, name="mh_ps")
                for h in range(HEADS):
                    b = (h % 2) * 64
                    pr = h // 2
                    nc.vector.reduce_max(
                        negmax[b:b + 64, pr:pr + 1],
                        dots_ps[pr][b:b + 64, b:b + 64],
                        axis=mybir.AxisListType.X, negate=True,
                    )
                for h in range(HEADS):
                    b = (h % 2) * 64
                    pr = h // 2
                    nc.scalar.activation(
                        exps[b:b + 64, pr * 64:(pr + 1) * 64],
                        dots_ps[pr][b:b + 64, b:b + 64],
                        mybir.ActivationFunctionType.Exp,
                        bias=negmax[b:b + 64, pr:pr + 1],
                        scale=1.0,
                        accum_out=rowsum[b:b + 64, pr:pr + 1],
                    )
                for pr in range(4):
                    nc.vector.reciprocal(
                        recip[:, pr:pr + 1], rowsum[:, pr:pr + 1]
                    )
                for pr in range(4):
                    nc.vector.tensor_scalar_mul(
                        wots[:, pr * 64:(pr + 1) * 64],
                        wot_sb[:, :],
                        recip[:, pr:pr + 1],
                    )
                for pr in range(4):
                    for s in range(2):
                        b = s * 64
                        nc.tensor.matmul(
                            mh_ps[b:b + 64, pr * 64:(pr + 1) * 64],
                            lhsT=exps[b:b + 64, pr * 64:(pr + 1) * 64],
                            rhs=wots[b:b + 64, pr * 64:(pr + 1) * 64],
                            start=True,
                            stop=True,
                        )
                for pr in range(4):
                    nc.vector.tensor_copy(
                        mh_sb[:, pr * 64:(pr + 1) * 64],
                        mh_ps[:, pr * 64:(pr + 1) * 64],
                    )

                # 2-bank fins PSUM tiles (2 chunks each) evacuated with one
                # copy; per-1024-col out tiles from a pool (distinct tiles
                # so the alternating DVE/ACT evacs don't serialize); each
                # 256KB chunk stores immediately on the idle sync/gpsimd
                # queues (a dma_start occupies its issuing engine ~600ns,
                # keep that off the evac engines).
                for pr in range(4):
                    for t2 in range(bl // 1024):
                        fp_ = finpool.tile([128, 1024], F32, name="fp_")
                        oc = outpool.tile([128, 1024], F16, name="oc")
                        for tt in range(2):
                            n0 = pr * bl + (2 * t2 + tt) * 512
                            c0 = tt * 512
                            nc.tensor.matmul(
                                fp_[0:64, c0:c0 + 512],
                                lhsT=mh_sb[0:64, pr * 64:(pr + 1) * 64],
                                rhs=vdp[0:64, n0:n0 + 512],
                                start=True,
                                stop=True,
                            )
                            nc.tensor.matmul(
                                fp_[64:128, c0:c0 + 512],
                                lhsT=mh_sb[64:128, pr * 64:(pr + 1) * 64],
                                rhs=vdp[64:128, n0:n0 + 512],
                                start=True,
                                stop=True,
                            )
                        if t2 % 2 == 0:
                            nc.vector.tensor_copy(oc[:, :], fp_[:, :])
                        else:
                            nc.scalar.copy(oc[:, :], fp_[:, :])
                        st_eng = nc.sync if t2 % 2 == 0 else nc.gpsimd
                        st_eng.dma_start(
                            out=out_ap[:, pr * bl + t2 * 1024:
                                       pr * bl + (t2 + 1) * 1024],
                            in_=oc[:, :])

    nc.compile()
    return nc


_NC_CACHE = {}


def _get_nc(hw=HW):
    if hw not in _NC_CACHE:
        _NC_CACHE[hw] = _build_kernel(hw)
    return _NC_CACHE[hw]


def _host_inputs(Wq, bq, Wk, bk, Wv, bv, Wo):
    scale = 64 ** -0.5
    wqkv = np.zeros((65, 192), np.float16)
    wqkv[:64, 0:64] = (Wq.T * scale).astype(np.float16)
    wqkv[64, 0:64] = (bq * scale).astype(np.float16)
    wqkv[:64, 64:128] = Wk.T.astype(np.float16)
    wqkv[64, 64:128] = bk.astype(np.float16)
    wqkv[:64, 128:192] = Wv.T.astype(np.float16)
    wqkv[64, 128:192] = bv.astype(np.float16)
    # kernel uses c' = blk*8 + chan ordering; original c = chan*8 + blk
    pi = np.array([(c % 8) * 8 + c // 8 for c in range(64)])
    wotp = Wo.T[pi]
    wot = np.concatenate([wotp, wotp], axis=0).astype(np.float32)
    ident = np.eye(128, dtype=np.float16)
    return wqkv, wot, ident


def kernel(x, Wq, bq, Wk, bk, Wv, bv, Wo):
    global LAST_RESULTS
    B = x.shape[0]
    hw = x.shape[2] * x.shape[3]
    nc = _get_nc(hw)
    wqkv, wot, ident = _host_inputs(Wq, bq, Wk, bk, Wv, bv, Wo)

    in_maps = []
    for bidx in range(B):
        x65 = np.empty((65, hw), np.float32)
        x65[:64] = x[bidx].reshape(64, hw)
        x65[64] = 1.0
        in_maps.append({"x": x65, "wqkv": wqkv, "wot": wot, "ident": ident})

    trace = bool(os.environ.get("KERNEL_TRACE"))
    res = run_bass_kernel_spmd(
        nc, in_maps, core_ids=list(range(B)), trace=trace
    )
    LAST_RESULTS = res
    bl = hw // HEADS
    # decode [128, hw/2] fp16 rows s*64+o, cols pr*bl+n -> [64, 8, bl] fp32
    out = np.stack(
        [res.results[bidx]["out"].reshape(2, 64, 4, bl)
         .transpose(1, 2, 0, 3).reshape(64, HEADS, bl).astype(np.float32)
         for bidx in range(B)]
    )
    return out
